# revision 37
# baseline (speedup 1.0000x reference)
"""3-layer GCN on 8 Trainium2 NeuronCores (Bass/Tile).

Distribution: nodes sharded contiguously across 8 cores (12500 each); edges
partitioned by dst core.  Per layer l:
  table g_l = norm_out * (h_l @ W_l.T)   (row-major fp16, built per-shard,
                                          AllGathered to every core's HBM)
  agg[d]   = b_l + sum_{e: dst=d} g_l[src_e]  (dma_gather by src + one-hot
                                          S-matmul segment-sum into PSUM;
                                          bias enters as a rank-1 matmul)
  h_{l+1}  = relu(agg * norm_in + h_l)   (last layer: no resid/relu)

dma_gather indices are int16, so the table is laid out CHUNK-major: 4 chunks
of node-tiles, each chunk holding all 8 cores' rows contiguously (< 32768
rows), so a gather window == a chunk.  Each chunk ends with a zero pad tile
per core for call-tail padding.  Gather calls rotate across the 4 SWDGE
queues ((w + sb) % 4), which lets the Q7 core pairs pipeline descriptor
generation 4-wide — desc-gen is the critical resource at ~7.3 ns/row/queue.

The table build for layer l+1 is folded into layer l's per-block epilogue
(transpose + W-matmul + norm scale on the scalar engine), and the table is
published with 4 chunked AllGathers that overlap the tail of layer l's
gathers.  Tables are double-buffered across layers so publishes never
conflict with in-flight gathers.

Self-contained: only numpy + concourse (the on-box bass stack).
"""

import numpy as np

N = 100000
D = 128
E = 1600000
NCORES = 8
SHARD = 12500          # nodes per core
NB = 98                # dst blocks of 128 per core (12544 slots, 44 dummies)
NSB = 13               # dst superblocks of 8 blocks (last has 2)
SB_BLOCKS = [list(range(sb * 8, min((sb + 1) * 8, NB))) for sb in range(NSB)]
SBATCH = 128           # S one-hot tiles streamed per DMA
NWIN = 4               # gather windows == table chunks
NREAL = [25, 25, 25, 23]           # real node-tiles per chunk (per core);
#                                    near-equal so the 4 SWDGE queues stay
#                                    load-balanced (in-order completion
#                                    starves on unequal call sizes)
NT_CH = [n + 1 for n in NREAL]     # +1 zero pad tile per chunk
B0_CH = [0, 25, 50, 75]            # first real block of each chunk
CH_T0 = [0, 26, 52, 78]            # stage-tile base of each chunk
ROWSPT = sum(NT_CH)                # stage tiles per core (102)
REGION = 128 * ROWSPT              # rows per core
WIN_ROWS = [8 * nt * 128 for nt in NT_CH]   # rows per window (< 32768)
CH_BASE = [0] + np.cumsum(WIN_ROWS)[:-1].tolist()
ZROW_W = [(nt - 1) * 128 for nt in NT_CH]   # window-relative zero row
TABLE_ROWS = NCORES * REGION
# stage tile of real block B
ST_OF_B = [CH_T0[min(B // 25, 3)] + B - B0_CH[min(B // 25, 3)]
           for B in range(NB)]
# chunk k's blocks all finish after superblock CHUNK_SB[k]-1; next-layer
# publishes fire one superblock later (slack so the collective trigger's
# sem wait never stalls the Pool sequencer mid-dispatch)
CHUNK_SB = [4, 7, 10, 13]
PUB_SB = [5, 8, 11, 13]


def _table_row(node):
    """Global table row of each node (chunk-major layout)."""
    node = np.asarray(node)
    c = node // SHARD
    i = node - c * SHARD
    t = i // 128
    k = np.minimum(t // 25, 3)
    t_in = t - np.array(B0_CH)[k]
    seg = np.array([nt * 128 for nt in NT_CH])
    base = np.array(CH_BASE)
    return base[k] + c * seg[k] + t_in * 128 + (i % 128)


def preprocess(src, dst):
    """Static schedule + per-core index data from the edge list."""
    src = np.asarray(src).astype(np.int64)
    dst = np.asarray(dst).astype(np.int64)

    deg_out = np.bincount(src, minlength=N).astype(np.float64)
    deg_in = np.bincount(dst, minlength=N).astype(np.float64)
    norm_out = np.clip(deg_out, 1.0, None) ** -0.5
    norm_in = np.clip(deg_in, 1.0, None) ** -0.5

    src_row = _table_row(src)
    src_t = (src - (src // SHARD) * SHARD) // 128
    win = np.minimum(src_t // 25, 3)     # window = chunk of src tile
    dst_core = dst // SHARD
    dst_local = dst - dst_core * SHARD
    dst_block = dst_local // 128
    dst_slot = dst_local % 128
    sb_of_block = np.arange(NB) // 8

    # sort edges by (core, superblock, window, block)
    key = (((dst_core * NSB + sb_of_block[dst_block]) * NWIN + win) * NB
           + dst_block)
    order = np.argsort(key, kind="stable")
    s_src_row = src_row[order]
    s_key = key[order]
    s_slot = dst_slot[order]

    # per-(core, sb, w, B) counts
    counts = np.zeros((NCORES, NSB, NWIN, NB), np.int64)
    uk, uc = np.unique(s_key, return_counts=True)
    kc = uk // (NSB * NWIN * NB)
    rem = uk % (NSB * NWIN * NB)
    ksb = rem // (NWIN * NB)
    rem = rem % (NWIN * NB)
    kw = rem // NB
    kb = rem % NB
    counts[kc, ksb, kw, kb] = uc

    call_edges = counts.sum(axis=3)                      # [NCORES, NSB, NWIN]
    call_tiles = (-(-call_edges // 128)).max(axis=0)     # [NSB, NWIN]
    # every block needs >=1 sub; guarantee the (sb, 0) call has >=1 tile
    for sb in range(NSB):
        if call_tiles[sb].sum() == 0:
            call_tiles[sb, 0] = 1

    # per-core cumulative start of each (sb, w, B) run inside its call
    run_start = np.cumsum(counts, axis=3) - counts       # [C, NSB, NWIN, NB]

    # union sub schedule: per call, per tile, blocks touched by ANY core
    subs = []           # per sub: (call_idx, tile_in_call, B)
    call_spans = []     # per call: (sub_lo, sub_hi, ntile)
    for sb in range(NSB):
        for w in range(NWIN):
            ntile = int(call_tiles[sb, w])
            sub_lo = len(subs)
            if ntile > 0:
                tile_blocks = [[] for _ in range(ntile)]
                for B in SB_BLOCKS[sb]:
                    touched = set()
                    for c in range(NCORES):
                        n = counts[c, sb, w, B]
                        if n == 0:
                            continue
                        t0 = int(run_start[c, sb, w, B]) // 128
                        t1 = int(run_start[c, sb, w, B] + n - 1) // 128
                        touched.update(range(t0, t1 + 1))
                    for t in sorted(touched):
                        tile_blocks[t].append(B)
                if sum(len(x) for x in tile_blocks) == 0:
                    tile_blocks[0] = list(SB_BLOCKS[sb])
                for t in range(ntile):
                    for B in tile_blocks[t]:
                        subs.append((len(call_spans), t, B))
            call_spans.append((sub_lo, len(subs), ntile))
    NSUB = len(subs)

    # blocks with no subs at all: add one sub in their superblock's (w=0)
    # call, tile 0
    have = set(B for _, _, B in subs)
    extra = []
    for B in range(NB):
        if B not in have:
            sb = B // 8
            ci = sb * NWIN
            extra.append((ci, 0, B))
    if extra:
        new_subs = []
        new_spans = []
        for ci, (lo, hi, ntile) in enumerate(call_spans):
            lo2 = len(new_subs)
            new_subs.extend(subs[lo:hi])
            new_subs.extend(x for x in extra if x[0] == ci)
            new_spans.append((lo2, len(new_subs), max(ntile, 1)))
        subs = new_subs
        call_spans = new_spans
        NSUB = len(subs)

    # start/stop flags per sub (first/last sub of each block)
    first_sub = {}
    last_sub = {}
    for j, (ci, t, B) in enumerate(subs):
        if B not in first_sub:
            first_sub[B] = j
        last_sub[B] = j
    flags = [(B, j == first_sub[B], j == last_sub[B])
             for j, (ci, t, B) in enumerate(subs)]

    # ---- per-core gather indices and per-sub dloc ----
    sub_lut = {}
    for j, (ci, t, B) in enumerate(subs):
        sub_lut[(ci, t, B)] = j

    ci_of = np.empty((NSB, NWIN), np.int64)
    for sb in range(NSB):
        for w in range(NWIN):
            ci_of[sb, w] = sb * NWIN + w
    tile_base = np.concatenate(
        [[0], np.cumsum([s[2] for s in call_spans])]).astype(np.int64)
    T_total = int(tile_base[-1])

    core_inputs = []
    for c in range(NCORES):
        k_lo = c * NSB * NWIN * NB
        k_hi = (c + 1) * NSB * NWIN * NB
        lo, hi = np.searchsorted(s_key, [k_lo, k_hi])
        ck = s_key[lo:hi] - k_lo
        csb = ck // (NWIN * NB)
        crem = ck % (NWIN * NB)
        cw = crem // NB
        cb = crem % NB
        crow = s_src_row[lo:hi]
        cslot = s_slot[lo:hi]
        # position within the call = run_start[c, sb, w, B] + rank in run
        pos_in_run = np.zeros(hi - lo, np.int64)
        if hi > lo:
            brk = np.flatnonzero(np.diff(ck) != 0) + 1
            starts = np.concatenate([[0], brk])
            lens = np.diff(np.concatenate([starts, [hi - lo]]))
            pos_in_run = np.arange(hi - lo) - np.repeat(starts, lens)
        pos_in_call = run_start[c, csb, cw, cb] + pos_in_run
        tile_in_call = pos_in_call // 128
        p_of_edge = pos_in_call % 128
        cci = ci_of[csb, cw]
        gtile = tile_base[cci] + tile_in_call

        idx16 = np.zeros((T_total, 128), np.int16)
        dloc = np.full((NSUB, 128), 255.0, np.float32)
        # defaults: every slot gathers its window's zero row
        for ci, (slo, shi, ntile) in enumerate(call_spans):
            idx16[tile_base[ci]:tile_base[ci + 1], :] = ZROW_W[ci % NWIN]
        idx16[gtile, p_of_edge] = (crow
                                   - np.array(CH_BASE)[cw]).astype(np.int16)
        sub_j = np.array([sub_lut[(int(a), int(b), int(d))]
                          for a, b, d in zip(cci, tile_in_call, cb)],
                         np.int64)
        dloc[sub_j, p_of_edge] = cslot.astype(np.float32)

        idx_d = np.zeros((128, T_total * 8), np.int16)
        for ci, (slo, shi, ntile) in enumerate(call_spans):
            if ntile == 0:
                continue
            t0 = int(tile_base[ci])
            flat = idx16[t0:t0 + ntile].reshape(ntile * 128)
            wrapped = flat.reshape(ntile * 8, 16).T
            idx_d[:, t0 * 8:(t0 + ntile) * 8] = np.tile(wrapped, (8, 1))
        # host-built one-hot S tiles (fp8): S[p, j*128 + s] = (dloc[j,p] == s)
        import ml_dtypes
        sm = (dloc[:, :, None] ==
              np.arange(128, dtype=np.float32)[None, None, :])
        smat = np.ascontiguousarray(
            sm.transpose(1, 0, 2).reshape(128, NSUB * 128)).astype(
                ml_dtypes.float8_e4m3fn)
        core_inputs.append((idx_d, smat))

    meta = dict(
        T_total=T_total, NSUB=NSUB, subs=subs, flags=flags,
        call_spans=call_spans, tile_base=tile_base,
        norm_out=norm_out.astype(np.float32),
        norm_in=norm_in.astype(np.float32),
    )
    return meta, core_inputs


def _slot_vec(vec):
    """[N] per-node vector -> per-core [128, NB] (pad nodes -> 0)."""
    out = []
    for c in range(NCORES):
        a = np.zeros(NB * 128, np.float32)
        a[:SHARD] = vec[c * SHARD:(c + 1) * SHARD]
        out.append(np.ascontiguousarray(a.reshape(NB, 128).T))
    return out


def _slot_rows(mat, dtype):
    """[N, D] rows -> per-core [128, NB*128] (h[p, B*D+f] = row of node
    c*SHARD + B*128 + p)."""
    out = []
    for c in range(NCORES):
        a = np.zeros((NB * 128, D), dtype)
        a[:SHARD] = mat[c * SHARD:(c + 1) * SHARD].astype(dtype)
        out.append(np.ascontiguousarray(
            a.reshape(NB, 128, D).transpose(1, 0, 2).reshape(128, NB * D)))
    return out


def build_program(meta):
    import concourse.mybir as mybir
    import concourse.tile as tile
    import concourse.bacc as bacc
    from concourse.masks import make_identity

    f16 = mybir.dt.float16
    f32 = mybir.dt.float32
    i16 = mybir.dt.int16
    f8 = mybir.dt.float8e4

    T_total = meta["T_total"]
    NSUB = meta["NSUB"]
    subs = meta["subs"]
    flags = meta["flags"]
    call_spans = meta["call_spans"]
    tile_base = meta["tile_base"]

    nc = bacc.Bacc("TRN2", target_bir_lowering=False, debug=False,
                   num_devices=NCORES, num_swdge_queues=4)

    h0_d = nc.dram_tensor("h0", [128, NB * D], f16, kind="ExternalInput")
    idx_d = nc.dram_tensor("gidx", [128, T_total * 8], i16,
                           kind="ExternalInput")
    smat_d = nc.dram_tensor("smat", [128, NSUB * 128], f8,
                            kind="ExternalInput")
    no_d = nc.dram_tensor("normout", [128, NB], f32, kind="ExternalInput")
    ni_d = nc.dram_tensor("normin", [128, NB], f32, kind="ExternalInput")
    wt_d = nc.dram_tensor("wt", [D, 3 * D], f16, kind="ExternalInput")
    bb4_d = nc.dram_tensor("bb4", [128, 3 * 4 * D], f16,
                           kind="ExternalInput")
    out_d = nc.dram_tensor("out", [128, NB * D], f16, kind="ExternalOutput")

    g_local = [nc.dram_tensor(f"g_local{i}", [REGION, D], f16,
                              kind="Internal") for i in range(2)]
    table = [nc.dram_tensor(f"gtable{i}", [TABLE_ROWS, D], f16,
                            kind="Internal", addr_space="Shared")
             for i in range(2)]

    relu = mybir.ActivationFunctionType.Relu
    copyf = mybir.ActivationFunctionType.Copy

    with tile.TileContext(nc) as tc:
        with (
            tc.tile_pool(name="const", bufs=1) as constp,
            tc.tile_pool(name="ht", bufs=3) as htp,
            tc.tile_pool(name="ix", bufs=10) as ixp,
            tc.tile_pool(name="msgs", bufs=9) as msgp,
            tc.tile_pool(name="sbu", bufs=2) as sp,
            tc.tile_pool(name="cc", bufs=6) as cp,
            tc.tile_pool(name="ps", bufs=4, space="PSUM") as psp,
            tc.tile_pool(name="psA", bufs=4, space="PSUM") as psap,
        ):
            ident = constp.tile([128, 128], f16)
            make_identity(nc, ident[:])
            ones_row = constp.tile([128, 128], f16)
            nc.vector.memset(ones_row[:], 0.0)
            nc.vector.memset(ones_row[0:1, :], 1.0)
            h_sb = constp.tile([128, NB * D], f16)
            nc.sync.dma_start(h_sb[:], h0_d.ap())
            no_sb = constp.tile([128, NB], f32)
            nc.sync.dma_start(no_sb[:], no_d.ap())
            ni_sb = constp.tile([128, NB], f32)
            nc.sync.dma_start(ni_sb[:], ni_d.ap())
            wt_sb = constp.tile([128, 3 * D], f16)
            nc.sync.dma_start(wt_sb[:], wt_d.ap())
            bb4_sb = constp.tile([128, 3 * 4 * D], f16)
            nc.sync.dma_start(bb4_sb[:], bb4_d.ap())
            stage = constp.tile([128, ROWSPT * D], f16)
            for k in range(NWIN):   # zero pad tile at the end of each chunk
                zt = CH_T0[k] + NT_CH[k] - 1
                nc.vector.memset(stage[:, zt * D:(zt + 1) * D], 0.0)

            def phase_a(l, B):
                """stage_B = norm_out * (h_B @ W_l.T) for the layer-l table."""
                psT = psap.tile([128, D], f16, tag="psA", name=f"psT{l}_{B}")
                nc.tensor.transpose(psT[:], h_sb[:, B * D:(B + 1) * D],
                                    ident[:])
                hT = htp.tile([128, D], f16, tag="hT", name=f"hT{l}_{B}")
                nc.vector.tensor_copy(hT[:], psT[:])
                psG = psap.tile([128, D], f32, tag="psA", name=f"psG{l}_{B}")
                nc.tensor.matmul(psG[:], lhsT=hT[:],
                                 rhs=wt_sb[:, l * D:(l + 1) * D],
                                 start=True, stop=True)
                st = ST_OF_B[B]
                nc.scalar.activation(stage[:, st * D:(st + 1) * D], psG[:],
                                     copyf, scale=no_sb[:, B:B + 1])

            def publish(l, k):
                """DMA stage chunk k to g_local and AllGather into table."""
                r0 = CH_T0[k] * 128
                r1 = r0 + NT_CH[k] * 128
                gl = g_local[l % 2]
                tb = table[l % 2]
                nc.scalar.dma_start(
                    gl.ap()[r0:r1, :].rearrange("(t p) d -> p t d", p=128),
                    stage[:, CH_T0[k] * D:(CH_T0[k] + NT_CH[k]) * D]
                    .rearrange("p (t d) -> p t d", d=D))
                nc.gpsimd.collective_compute(
                    "AllGather", mybir.AluOpType.bypass,
                    replica_groups=[list(range(NCORES))],
                    ins=[gl.ap()[r0:r1, :]],
                    outs=[tb.ap()[CH_BASE[k]:CH_BASE[k] + WIN_ROWS[k], :]],
                )

            # layer-0 table build upfront, publishing chunks as they finish
            pubk = 0
            pub_after = [25, 50, 75, 98]
            for B in range(NB):
                phase_a(0, B)
                while pubk < NWIN and B + 1 >= pub_after[pubk]:
                    publish(0, pubk)
                    pubk += 1

            for l in range(3):
                tb = table[l % 2]
                psum_of = {}
                psgrp = {}
                next_chunk = 0
                for ci, (slo, shi, ntile) in enumerate(call_spans):
                    if ntile == 0:
                        continue
                    w = ci % NWIN
                    sb = ci // NWIN
                    ni_call = ntile * 128
                    t0 = int(tile_base[ci])
                    ixt = ixp.tile([128, ntile * 8], i16, tag="ix",
                                   name=f"ix{l}_{ci}")
                    nc.sync.dma_start(ixt[:],
                                      idx_d.ap()[:, t0 * 8:(t0 + ntile) * 8])
                    msgs = msgp.tile([128, ntile * D], f16, tag="m",
                                     name=f"m{l}_{ci}")
                    nc.gpsimd.dma_gather(
                        out_ap=msgs[:].rearrange("p (t d) -> p t d", d=D),
                        in_ap=tb.ap()[CH_BASE[w]:CH_BASE[w] + WIN_ROWS[w], :],
                        idxs_ap=ixt[:],
                        num_idxs=ni_call,
                        num_idxs_reg=ni_call,
                        elem_size=D,
                        single_packet=(ni_call <= 1024),
                        queue_num=(w + sb) % NWIN,
                    )
                    for j in range(slo, shi):
                        _, t, B = subs[j]
                        _, is_first, is_last = flags[j]
                        k = j % SBATCH
                        if k == 0:
                            s0 = j
                            nb2 = min(SBATCH, NSUB - s0)
                            sb_t = sp.tile([128, SBATCH * 128], f8, tag="S",
                                           name=f"S{l}_{j}")
                            nc.sync.dma_start(
                                sb_t[:, :nb2 * 128],
                                smat_d.ap()[:, s0 * 128:(s0 + nb2) * 128])
                            sbatch_tile = sb_t
                        St = sbatch_tile[:, k * 128:(k + 1) * 128]
                        if is_first:
                            # 4 block-aggregators share one PSUM bank; init
                            # the whole bank ONCE with the rank-1 bias
                            # (start=True clears accumulate bits bank-wide)
                            gkey = (B // 8, (B % 8) // 4)
                            if gkey not in psgrp:
                                psgrp[gkey] = psp.tile(
                                    [128, 4 * D], f32, tag="ps",
                                    name=f"agg{l}_{gkey[0]}_{gkey[1]}")
                                nc.tensor.matmul(
                                    psgrp[gkey][:],
                                    lhsT=ones_row[:],
                                    rhs=bb4_sb[:, l * 4 * D:(l + 1) * 4 * D],
                                    start=True, stop=False)
                            q = (B % 4) * D
                            psum_of[B] = psgrp[gkey][:, q:q + D]
                        nc.tensor.matmul(psum_of[B], lhsT=St,
                                         rhs=msgs[:, t * D:(t + 1) * D],
                                         start=False, stop=is_last)
                        if is_last:
                            # epilogue for block B
                            pa = psum_of.pop(B)
                            if l < 2:
                                x2 = cp.tile([128, D], f16, tag="x2",
                                             name=f"x2{l}_{B}")
                                nc.scalar.activation(
                                    x2[:], pa, copyf,
                                    scale=ni_sb[:, B:B + 1])
                                x3 = cp.tile([128, D], f16, tag="x3",
                                             name=f"x3{l}_{B}")
                                nc.vector.tensor_add(
                                    x3[:], x2[:], h_sb[:, B * D:(B + 1) * D])
                                nc.scalar.activation(
                                    h_sb[:, B * D:(B + 1) * D], x3[:], relu)
                                phase_a(l + 1, B)
                            else:
                                st = ST_OF_B[B]
                                nc.scalar.activation(
                                    stage[:, st * D:(st + 1) * D], pa,
                                    copyf, scale=ni_sb[:, B:B + 1])
                    # publish next-layer table chunks / output as ready
                    sb_done = sb + (1 if w == NWIN - 1 else 0)
                    if l < 2:
                        while (next_chunk < len(PUB_SB)
                               and sb_done >= PUB_SB[next_chunk]):
                            publish(l + 1, next_chunk)
                            next_chunk += 1
                    else:
                        while (next_chunk < len(CHUNK_SB)
                               and sb_done >= CHUNK_SB[next_chunk]):
                            k2 = next_chunk
                            b0 = B0_CH[k2]
                            nc.scalar.dma_start(
                                out_d.ap()[:, b0 * D:(b0 + NREAL[k2]) * D],
                                stage[:, CH_T0[k2] * D:
                                      (CH_T0[k2] + NREAL[k2]) * D])
                            next_chunk += 1

    nc.compile()
    return nc


_CACHE = {}


def kernel(feat, src, dst, W1, b1, W2, b2, W3, b3):
    import hashlib
    import concourse.bass_utils as bass_utils

    feat = np.asarray(feat, np.float32)
    src = np.asarray(src)
    dst = np.asarray(dst)
    key = hashlib.sha1(src.tobytes() + dst.tobytes()).hexdigest()
    if key not in _CACHE:
        meta, core_inputs = preprocess(src, dst)
        nc = build_program(meta)
        _CACHE[key] = (meta, core_inputs, nc)
    meta, core_inputs, nc = _CACHE[key]

    Wt = np.concatenate([np.asarray(w, np.float32).T for w in (W1, W2, W3)],
                        axis=1).astype(np.float16)          # [D, 3D]
    # bias tiled 4x per layer (one rank-1 matmul initializes a whole
    # 4-block PSUM bank)
    bb4 = np.tile(np.concatenate(
        [np.tile(np.asarray(b, np.float32), 4) for b in (b1, b2, b3)])[None,
                                                                       :],
        (128, 1)).astype(np.float16)                         # [128, 12D]

    h0_cores = _slot_rows(feat, np.float16)
    no_cores = _slot_vec(meta["norm_out"])
    ni_cores = _slot_vec(meta["norm_in"])

    in_maps = []
    for c in range(NCORES):
        idx_d, smat = core_inputs[c]
        in_maps.append({
            "h0": h0_cores[c],
            "gidx": idx_d,
            "smat": smat,
            "normout": no_cores[c],
            "normin": ni_cores[c],
            "wt": Wt,
            "bb4": bb4,
        })

    res = bass_utils.run_bass_kernel_spmd(nc, in_maps,
                                          core_ids=list(range(NCORES)))
    out = np.zeros((N, D), np.float32)
    for c in range(NCORES):
        o = res.results[c]["out"].astype(np.float32)
        rows = o.reshape(128, NB, D).transpose(1, 0, 2).reshape(NB * 128, D)
        out[c * SHARD:(c + 1) * SHARD] = rows[:SHARD]
    return out
